# revision 24
# baseline (speedup 1.0000x reference)
"""Trainium2 Bass kernel for nn_Cross_attention2 (dense transformer cross-attention).

Math (per batch b, head h), faithful to the reference module (which uses the
fc_q weights W_h for Q, K AND V):
    Q = q W_h + b_h ; K = k W_h + b_h ; V = v W_h + b_h
    alpha = (Q K^T)/sqrt(512); masked -> -1e9; alpha /= sqrt(512); P = softmax(alpha)
    out[b, :, h*512:(h+1)*512] = P @ V

Device algorithm (algebraically identical post-softmax), all four big matmul
groups in fp8-e4m3 DoubleRow mode (2x PE rate, fp32 PSUM accumulate):
    G    = W_h W_h^T, Wb = W_h b_h            (host fp32 -> fp8)
    Z    = G q^T + Wb 1^T                     [512, Lq]   (k-varying score term;
             per-q and constant terms drop under softmax)
    s^T  = k Z ; P = exp(s^T/512 [+ mb/512])  (scores ~N(0, (1/23)^2): no max-sub)
    Phat = P - 1                              (ACT Exp -> fp16, DVE sub -> fp8:
             quantizing P~1.0 directly would bury the attention signal in
             fp8 rounding; Phat keeps the error ~6% * |Phat| ~ 3e-3)
    sums = 512 + colsum(Phat)                 (tiny DR ones-matmuls + reciprocal)
    Onum = Phat^T (v W_h)                     (device, fp8 DR)
    out  = Onum/sums  + [colsum(v) W_h]/sums + b_h
           ^ device: (psum * rsb) per-partition, fp16 out
                      ^ host epilogue: exact rank-1 colsum term + bias applied
                        during unshard (device exports 1/sums per (b,h))
Per pair (b,h) the PE does 32 DoubleRow MMs (512-contraction each) + 8 tiny
sums MMs: the 4*512^3-MAC algorithmic floor at the fp8 rate. psum->sbuf
copies alternate DVE/ACT; both heads are phase-interleaved so copy chains
have an 8-MM PE window. HAM warmup dummies + striped dual-queue (sync+scalar
HWDGE) first-batch DMAs cover the fixed ~7us engine-preamble.
Sharding: 2 batch-groups x 4 head-groups; 8 batches x 2 heads per core.
Masked variant (any mask entry == 0) compiles lazily; fully-masked query rows
are degenerate in the reference too and are not supported.
"""

import os
import sys
from contextlib import ExitStack

import numpy as np
import ml_dtypes

for _p in ("/opt/trn_rl_repo",):
    if os.path.isdir(_p) and _p not in sys.path:
        sys.path.append(_p)

import concourse.bacc as bacc
import concourse.mybir as mybir
import concourse.tile as tile
from concourse.bass import ts
from concourse.bass_utils import run_bass_kernel_spmd

dt = mybir.dt
F8 = ml_dtypes.float8_e4m3

B, L, D, H = 16, 512, 512, 8
NCORES = 8
BGROUPS, HGROUPS = 2, 4          # core grid: 2 batch-groups x 4 head-groups
BPC = B // BGROUPS               # 8 batches per core
HPC = H // HGROUPS               # 2 heads per core
C = D // 128                     # 128-row chunks per 512
NEG_MASK = -1e9 * float(np.sqrt(512.0))  # additive bias for masked entries (pre /512)

_CACHE = {}


def _build(masked: bool):
    nc = bacc.Bacc("TRN2", target_bir_lowering=False, debug=False, num_devices=NCORES)
    f32 = dt.float32
    f16 = dt.float16
    f8 = dt.float8e4
    DR = mybir.MatmulPerfMode.DoubleRow
    EXP = mybir.ActivationFunctionType.Exp
    COPY = mybir.ActivationFunctionType.Copy
    MUL = mybir.AluOpType.mult
    ADD = mybir.AluOpType.add

    qT_d = nc.dram_tensor("qT", [BPC, D, L], f8, kind="ExternalInput").ap()
    kT_d = nc.dram_tensor("kT", [BPC, D, L], f8, kind="ExternalInput").ap()
    vT_d = nc.dram_tensor("vT", [BPC, D, L], f8, kind="ExternalInput").ap()
    G_d = nc.dram_tensor("G", [HPC, D, D], f8, kind="ExternalInput").ap()
    W_d = nc.dram_tensor("W", [HPC, D, D], f8, kind="ExternalInput").ap()
    Wb_d = nc.dram_tensor("Wb", [HPC, 128, C], f32, kind="ExternalInput").ap()
    ones8_d = nc.dram_tensor("ones8", [128, C, 2], f8, kind="ExternalInput").ap()
    rsb_d = nc.dram_tensor("rsb", [BPC, HPC, 128, 8], f32, kind="ExternalOutput").ap()
    if masked:
        mbT_d = nc.dram_tensor("mbT", [BPC, L, L], f32, kind="ExternalInput").ap()
    out_d = nc.dram_tensor("out", [BPC, L, HPC * D], f16, kind="ExternalOutput").ap()

    with tile.TileContext(nc) as tc, ExitStack() as ctx:
        const = ctx.enter_context(tc.tile_pool(name="const", bufs=1))
        headp = ctx.enter_context(tc.tile_pool(name="headp", bufs=1))
        acts = ctx.enter_context(tc.tile_pool(name="acts", bufs=3))
        work = ctx.enter_context(tc.tile_pool(name="work", bufs=2))
        psb = ctx.enter_context(tc.tile_pool(name="psb", bufs=4, space="PSUM"))
        pso = ctx.enter_context(tc.tile_pool(name="pso", bufs=3, space="PSUM"))
        pss = ctx.enter_context(tc.tile_pool(name="pss", bufs=1, space="PSUM"))

        # ---- HAM warmup: dummy matmuls on memset scratch keep the PE busy
        # from t~0 so the clock-gate is at 8/8 by the time q[0] lands ----
        scratch = const.tile([128, 2, L], f8, tag="scr")
        nc.vector.memset(scratch[:], 0.0)
        wps = pso.tile([128, L], f32, tag="o", name="warm")
        for _ in range(10):
            nc.tensor.matmul(
                wps[:], scratch[:, :, 0:128], scratch[:],
                start=True, stop=True, perf_mode=mybir.MatmulPerfMode.DoubleRow,
            )

        # ---- weight/constant loads (PE's first MMs need G[0] + q[0] only;
        # b=0 loads are split into chunk-pairs so Z/s can start early) ----
        Gs = [headp.tile([128, C, D], f8, tag=f"G{h}", name=f"Gs{h}") for h in range(HPC)]

        def load_q(b, strips=1):
            tq = acts.tile([128, C, L], f8, tag="q", name=f"qTs{b}")
            for j in range(strips):
                w = C // strips
                eng = nc.scalar if (strips > 1 and j % 2 == 1) else nc.sync
                eng.dma_start(
                    tq[:, j * w : (j + 1) * w, :],
                    qT_d[b, j * w * 128 : (j + 1) * w * 128].rearrange(
                        "(c p) q -> p c q", p=128
                    ),
                )
            return tq

        def load_kvm(b, tq, strips=1):
            tk = acts.tile([128, C, L], f8, tag="k", name=f"kTs{b}")
            tv = acts.tile([128, C, L], f8, tag="v", name=f"vTs{b}")
            for tt, dd in ((tk, kT_d), (tv, vT_d)):
                for j in range(strips):
                    w = C // strips
                    eng = nc.scalar if (b == 0 and j % 2 == 1) else nc.sync
                    eng.dma_start(
                        tt[:, j * w : (j + 1) * w, :],
                        dd[b, j * w * 128 : (j + 1) * w * 128].rearrange(
                            "(c p) q -> p c q", p=128
                        ),
                    )
            if masked:
                tm = acts.tile([128, C, L], f32, tag="m", name=f"mbs{b}")
                nc.sync.dma_start(tm[:], mbT_d[b].rearrange("(c p) q -> p c q", p=128))
            else:
                tm = None
            return (tq, tk, tv, tm)

        # interleaved chunk-pair loads: G0/q0 pair0 first (first Z matmul),
        # then pair1, then k0 for the s matmuls
        nc.sync.dma_start(
            Gs[0][:, 0:2, :], G_d[0, 0:256].rearrange("(c p) d -> p c d", p=128)
        )
        _tq0 = load_q(0, strips=4)
        nc.sync.dma_start(
            Gs[0][:, 2:4, :], G_d[0, 256:512].rearrange("(c p) d -> p c d", p=128)
        )
        cur_acts = load_kvm(0, _tq0, strips=2)
        nc.sync.dma_start(Gs[1][:], G_d[1].rearrange("(c p) d -> p c d", p=128))
        Ws, Wbs = [], []
        for h in range(HPC):
            w = headp.tile([128, C, D], f8, tag=f"W{h}", name=f"Ws{h}")
            nc.sync.dma_start(w[:], W_d[h].rearrange("(c p) d -> p c d", p=128))
            Ws.append(w)
            wb = headp.tile([128, C], f32, tag=f"Wb{h}", name=f"Wbs{h}")
            nc.sync.dma_start(wb[:], Wb_d[h])
            Wbs.append(wb)
        ones8 = const.tile([128, C, 2], f8, tag="ones8")
        nc.sync.dma_start(ones8[:], ones8_d)

        def emit_sums_O(st, final=False):
            """sums + O for a finished (b, h). The tiny sums MMs run first so
            1/sums is ready before the O MMs drain; each O tile then needs only
            [2 DR MMs] -> (psum * rsb) -> fp16 DMA, so PSUM banks free
            progressively. The colsum(v)W/sums + b terms are host-applied."""
            PT8, V8, b, h = st
            sums = pss.tile([128, 8], f32, tag="sums")
            srec = work.tile([128, 8], f32, tag="srec")
            rsb = work.tile([128, 8], f32, tag="rsb")
            n = 0
            for u in range(C):
                for cp in range(C // 2):
                    nc.tensor.matmul(
                        sums[:, 2 * u : 2 * u + 2],
                        PT8[:, 2 * cp : 2 * cp + 2, ts(u, 128)],
                        ones8[:, 2 * cp : 2 * cp + 2, :],
                        start=(n == 0), stop=(n == C * (C // 2) - 1),
                        perf_mode=DR, skip_group_check=True,
                    )
                    n += 1
            nc.vector.tensor_scalar_add(srec[:], sums[:], 512.0)
            nc.vector.reciprocal(rsb[:], srec[:])
            nc.sync.dma_start(rsb_d[b, h], rsb[:])
            for u in range(C):
                ops = pso.tile([128, D], f32, tag="o", name=f"ops{u}")
                for cp in range(C // 2):
                    nc.tensor.matmul(
                        ops[:],
                        PT8[:, 2 * cp : 2 * cp + 2, ts(u, 128)],
                        V8[:, 2 * cp : 2 * cp + 2, :],
                        start=(cp == 0), stop=(cp == C // 2 - 1), perf_mode=DR,
                    )
                Osb = work.tile([128, D], f16, tag="O", bufs=3, name=f"Osb{u}")
                if final and u % 2 == 1:
                    nc.scalar.mul(Osb[:], ops[:], rsb[:, 2 * u : 2 * u + 1])
                else:
                    nc.vector.tensor_scalar_mul(Osb[:], ops[:], rsb[:, 2 * u : 2 * u + 1])
                split = 2 if final else 1
                w = D // split
                deng = nc.sync if u % 2 == 0 else nc.scalar
                for j in range(split):
                    deng.dma_start(
                        out_d[b, ts(u, 128), h * D + j * w : h * D + (j + 1) * w],
                        Osb[:, j * w : (j + 1) * w],
                    )

        # Software pipeline over batches; both heads are phase-interleaved so
        # every psum->sbuf copy chain has a full 8-MM window of PE work before
        # its consumer needs it (hides DVE/ACT latency + semaphore hops).
        pend = []
        nxt_acts = load_kvm(1, load_q(1))
        for b in range(BPC):
            qTb, kTb, vTb, mbb = cur_acts
            if b + 2 < BPC:
                nxt2_acts = load_kvm(b + 2, load_q(b + 2))

            # Z = G q^T (+ Wb per-partition bias on the psum->sbuf copy);
            # copies alternate DVE/ACT so the copy chain keeps up with PE
            Zs = []
            for h in range(HPC):
                Zsb = work.tile([128, C, L], f8, tag="Z")
                if b == 0:
                    # cp-outer: the first 4 MMs need only the first chunk-pair
                    # strip of q/G, so PE starts before the second strip lands
                    zpss = [psb.tile([128, L], f32, tag="big", name=f"zp{t}") for t in range(C)]
                    for cp in range(C // 2):
                        for t in range(C):
                            nc.tensor.matmul(
                                zpss[t][:], Gs[h][:, 2 * cp : 2 * cp + 2, ts(t, 128)],
                                qTb[:, 2 * cp : 2 * cp + 2, :],
                                start=(cp == 0), stop=(cp == C // 2 - 1), perf_mode=DR,
                            )
                    for t in range(C):
                        if (h * C + t) % 2 == 0:
                            nc.vector.tensor_scalar_add(Zsb[:, t, :], zpss[t][:], Wbs[h][:, t : t + 1])
                        else:
                            nc.scalar.add(Zsb[:, t, :], zpss[t][:], Wbs[h][:, t : t + 1])
                else:
                    for t in range(C):
                        zps = psb.tile([128, L], f32, tag="big")
                        for cp in range(C // 2):
                            nc.tensor.matmul(
                                zps[:], Gs[h][:, 2 * cp : 2 * cp + 2, ts(t, 128)],
                                qTb[:, 2 * cp : 2 * cp + 2, :],
                                start=(cp == 0), stop=(cp == C // 2 - 1), perf_mode=DR,
                            )
                        if (h * C + t) % 2 == 0:
                            nc.vector.tensor_scalar_add(Zsb[:, t, :], zps[:], Wbs[h][:, t : t + 1])
                        else:
                            nc.scalar.add(Zsb[:, t, :], zps[:], Wbs[h][:, t : t + 1])
                Zs.append(Zsb)

            # previous batch's first pair drains here (fills the Z->s gap)
            if pend:
                emit_sums_O(pend.pop(0))

            # s^T = k Z ; Phat = exp(s^T/512 [+ mb/512]) - 1
            # h-interleaved: the kT stationary slice is shared by both heads,
            # so consecutive MMs reuse the same weights
            PTs = [work.tile([128, C, L], f8, tag="PT", bufs=4, name=f"PT{h}") for h in range(HPC)]
            for t in range(C):
                spss = [psb.tile([128, L], f32, tag="big", name=f"sp{h}") for h in range(HPC)]
                for cp in range(C // 2):
                    for h in range(HPC):
                        nc.tensor.matmul(
                            spss[h][:], kTb[:, 2 * cp : 2 * cp + 2, ts(t, 128)],
                            Zs[h][:, 2 * cp : 2 * cp + 2, :],
                            start=(cp == 0), stop=(cp == C // 2 - 1), perf_mode=DR,
                        )
                for h in range(HPC):
                    sm = work.tile([128, L], f16, tag="sm", bufs=4, name=f"sm{t}{h}")
                    if masked:
                        smm = work.tile([128, L], f32, tag="smm", bufs=2, name=f"smm{t}{h}")
                        nc.vector.tensor_add(smm[:], spss[h][:], mbb[:, t, :])
                        nc.scalar.activation(sm[:], smm[:], EXP, scale=1.0 / 512.0)
                    else:
                        nc.scalar.activation(sm[:], spss[h][:], EXP, scale=1.0 / 512.0)
                    nc.vector.tensor_scalar_sub(PTs[h][:, t, :], sm[:], 1.0)

            # previous batch's second pair drains here
            if pend:
                emit_sums_O(pend.pop(0))

            # V = v W  (bias + colsum handled host-side via T'); copies
            # alternate ACT/DVE
            Vs = []
            for h in range(HPC):
                V8 = work.tile([128, C, D], f8, tag="V", bufs=4)
                for t in range(C):
                    vps = psb.tile([128, D], f32, tag="big")
                    for cp in range(C // 2):
                        nc.tensor.matmul(
                            vps[:], vTb[:, 2 * cp : 2 * cp + 2, ts(t, 128)],
                            Ws[h][:, 2 * cp : 2 * cp + 2, :],
                            start=(cp == 0), stop=(cp == C // 2 - 1), perf_mode=DR,
                        )
                    if (h * C + t) % 2 == 1:
                        nc.vector.tensor_copy(V8[:, t, :], vps[:])
                    else:
                        nc.scalar.activation(V8[:, t, :], vps[:], COPY)
                Vs.append(V8)
                # last batch: drain pair h right away so its mul/DMA tail
                # overlaps the other head's V matmuls
                if b == BPC - 1:
                    emit_sums_O((PTs[h], Vs[h], b, h), final=True)

            if b < BPC - 1:
                pend += [(PTs[0], Vs[0], b, 0), (PTs[1], Vs[1], b, 1)]
            if b + 1 < BPC:
                cur_acts = nxt_acts
            if b + 2 < BPC:
                nxt_acts = nxt2_acts

    nc.compile()
    return nc


def _prep_inputs(query, key, value, mask, Wq, bq):
    f = np.float32

    def c8(x):  # TRN e4m3 (ml_dtypes.float8_e4m3 matches; clip to max normal)
        return np.clip(np.asarray(x, f), -240.0, 240.0).astype(F8)

    qT = np.ascontiguousarray(c8(np.asarray(query, f).transpose(0, 2, 1)))
    kT = np.ascontiguousarray(c8(np.asarray(key, f).transpose(0, 2, 1)))
    vT = np.ascontiguousarray(c8(np.asarray(value, f).transpose(0, 2, 1)))
    W32 = np.asarray(Wq, f)
    b32 = np.asarray(bq, f)
    G8 = c8(np.einsum("hde,hfe->hdf", W32, W32))            # [H, D, D]
    W8 = c8(W32)
    Wb = np.einsum("hde,he->hd", W32, b32)                   # [H, D]
    WbCol = np.ascontiguousarray(Wb.reshape(H, C, 128).transpose(0, 2, 1), f)  # [H,128,C]
    vsum = np.asarray(value, f).sum(axis=1)                  # [B, D]
    Tp = np.einsum("bd,hde->bhe", vsum, W32)                 # [B, H, D] fp32, host-applied
    bb = np.broadcast_to(b32[:, None, :], (H, 128, D)).copy()
    ones8 = np.ones((128, C, 2), F8)
    one16 = np.ones((1, 128), np.float16)

    m = np.asarray(mask)
    masked = not bool((m != 0).all())
    if masked:
        mb = (1.0 - (m != 0).astype(f)) * f(NEG_MASK)
        mbT = np.ascontiguousarray(mb.transpose(0, 2, 1))

    in_maps = []
    for c in range(NCORES):
        gb, gh = divmod(c, HGROUPS)
        bs = slice(gb * BPC, (gb + 1) * BPC)
        hs = slice(gh * HPC, (gh + 1) * HPC)
        im = {
            "qT": qT[bs], "kT": kT[bs], "vT": vT[bs],
            "G": np.ascontiguousarray(G8[hs]),
            "W": np.ascontiguousarray(W8[hs]),
            "Wb": np.ascontiguousarray(WbCol[hs]),
            "bb": np.ascontiguousarray(bb[hs]),
            "ones8": ones8, "one16": one16,
        }
        if masked:
            im["mbT"] = mbT[bs]
        in_maps.append(im)
    return in_maps, masked, Tp, b32


def _run(inputs, trace=False):
    in_maps, masked, Tp, _B32 = _prep_inputs(**inputs)
    key = "ncm" if masked else "nc"
    if key not in _CACHE:
        _CACHE[key] = _build(masked)
    nc = _CACHE[key]
    last_err = None
    for _attempt in range(3):
        try:
            res = run_bass_kernel_spmd(
                nc, in_maps, core_ids=list(range(NCORES)), trace=trace
            )
            break
        except Exception as e:  # transient NRT device errors happen; retry
            last_err = e
    else:
        raise last_err
    out = np.empty((B, L, H * D), np.float32)
    for c in range(NCORES):
        gb, gh = divmod(c, HGROUPS)
        blk = res.results[c]["out"].astype(np.float32)  # [BPC, L, HPC*D]
        rsb = res.results[c]["rsb"]  # [BPC, HPC, 128, 8]
        # recip[q] for q = u*128 + p lives at rsb[b, h, p, 2u]
        recip = rsb[:, :, :, 0::2].transpose(0, 1, 3, 2).reshape(BPC, HPC, L)
        blk = blk.reshape(BPC, L, HPC, D) + (
            Tp[gb * BPC : (gb + 1) * BPC, gh * HPC : (gh + 1) * HPC][:, :, None, :]
            * recip[:, :, :, None]
            + _B32[None, gh * HPC : (gh + 1) * HPC, None, :]
        ).transpose(0, 2, 1, 3)
        out[gb * BPC : (gb + 1) * BPC, :, gh * HPC * D : (gh + 1) * HPC * D] = (
            blk.reshape(BPC, L, HPC * D)
        )
    return out, res


def kernel(**inputs) -> np.ndarray:
    out, _ = _run(inputs, trace=False)
    return out
